# revision 11
# baseline (speedup 1.0000x reference)
"""2-layer GAT (DGL GATConv eval-mode) on 8 Trainium2 NeuronCores.

Strategy:
  - Nodes are partitioned into 8 blocks of 2500 (padded to 2560 = 20 windows
    of 128). Core m owns block m: it computes the dense projections for its
    own nodes and aggregates messages for destination nodes in its block.
  - Per layer:
    Phase A (dense, sharded): h|el|er = x_blk @ [W | W@al | W@ar] via
      TensorE (bf16), written as a 384-col bf16 node table
      [h(256) | el(8) | er(8) | pad(112)] -> 768B rows; AllGather -> full
      table on every core.
    Phase B (edges, dst-sharded): edges sorted by dst, grouped into
      128-dst windows, chunks of 128 edges. Per window: one dma_gather of
      full 768B rows by src (gives h and el per edge), one 256B dma_gather
      of [el|er|pad] by local dst (gives er per edge; reads the local
      pre-AG table). ex = exp(leaky_relu(el_src + er_dst)). One-hot
      (iota == dstloc) matmuls accumulate both sum(ex*h) and sum(ex) into
      PSUM in a single 264-wide matmul per 128-edge chunk. Normalize by
      1/sum(ex), add bias, ELU.
  - Layer-1 outputs are PE-transposed into an SBUF-resident x2T so layer 2's
    dense phase needs no DMA for activations.
"""
import sys

for _p in ("/opt/trn_rl_repo",):
    if _p not in sys.path:
        sys.path.append(_p)

import numpy as np

N = 20000
E = 320000
H = 8
D = 32
HD = H * D          # 256
NEG = 0.2
P = 128
NCORES = 8
BLK = 2500          # real nodes per core block
NWIN = 20           # windows per core
BP = NWIN * P       # padded block stride 2560
NB = NCORES * BP    # padded global rows 20480
TBL_W = 384         # table row: h 0:256 | el 256:264 | er 264:272 | pad
ELR_OFF = 256       # er-gather reads cols [256:384]; er at relative 8:16

_BUILD_CACHE = {}


def _host_prep(features, W0, al0, ar0, b0, W1, al1, ar1, b1, src, dst):
    import ml_dtypes
    bf16 = ml_dtypes.bfloat16

    def wcat(W, al, ar):
        Wal = np.stack([W[:, h * D:(h + 1) * D] @ al[h] for h in range(H)], axis=1)
        War = np.stack([W[:, h * D:(h + 1) * D] @ ar[h] for h in range(H)], axis=1)
        return np.ascontiguousarray(
            np.concatenate([W, Wal, War], axis=1).astype(bf16))  # [256, 272]

    wc0 = wcat(W0, al0, ar0)
    wc1 = wcat(W1, al1, ar1)
    b0r = np.ascontiguousarray(np.tile(b0[None, :], (P, 1)).astype(np.float32))
    b1r = np.ascontiguousarray(np.tile(b1[None, :], (P, 1)).astype(np.float32))
    iota = np.ascontiguousarray(
        np.tile(np.arange(P, dtype=np.float32)[None, :], (P, 1)))
    ident = np.eye(P, dtype=np.float32).astype(bf16)

    # per-core transposed, block-padded features [256, BP]
    xT = np.zeros((NCORES, HD, BP), dtype=bf16)
    xf = features.astype(bf16)
    for m in range(NCORES):
        xT[m, :, :BLK] = xf[m * BLK:(m + 1) * BLK].T

    # ---- edge partitioning ----
    owner = dst // BLK
    dst_local = dst - owner * BLK          # [0, 2500)
    win = dst_local // P                   # [0, 20)
    src_blocked = (src // BLK) * BP + (src % BLK)

    # per (core, window) edge lists
    counts = np.zeros((NCORES, NWIN), np.int64)
    lists = [[None] * NWIN for _ in range(NCORES)]
    for m in range(NCORES):
        sel = np.nonzero(owner == m)[0]
        w_m = win[sel]
        order = np.argsort(w_m, kind="stable")
        sel = sel[order]
        w_m = w_m[order]
        bounds = np.searchsorted(w_m, np.arange(NWIN + 1))
        for w in range(NWIN):
            ee = sel[bounds[w]:bounds[w + 1]]
            lists[m][w] = ee
            counts[m, w] = len(ee)

    C_ws = [int(np.ceil(counts[:, w].max() / P)) for w in range(NWIN)]
    CT = int(np.sum(C_ws))
    GC8 = CT * 8

    srcg = np.zeros((NCORES, P, GC8), np.int16)
    dstg = np.zeros((NCORES, P, GC8), np.int16)
    dloc = np.full((NCORES, P, CT), -1.0, np.float32)

    for m in range(NCORES):
        woff = 0
        woff8 = 0
        for w in range(NWIN):
            Cw = C_ws[w]
            L = Cw * P
            ee = lists[m][w]
            sb = np.zeros(L, np.int16)
            db = np.zeros(L, np.int16)
            dl = np.full(L, -1.0, np.float32)
            n = len(ee)
            sb[:n] = src_blocked[ee]
            db[:n] = dst_local[ee]
            dl[:n] = (dst_local[ee] - w * P).astype(np.float32)
            # gather idx layout: index i at [i%16, i//16], replicated x8 down
            sg = sb.reshape(Cw * 8, 16).T      # [16, Cw*8]
            dg = db.reshape(Cw * 8, 16).T
            srcg[m, :, woff8:woff8 + Cw * 8] = np.tile(sg, (8, 1))
            dstg[m, :, woff8:woff8 + Cw * 8] = np.tile(dg, (8, 1))
            # dstloc layout: edge i at [i%128, i//128]
            dloc[m, :, woff:woff + Cw] = dl.reshape(Cw, P).T
            woff += Cw
            woff8 += Cw * 8

    in_maps = []
    for m in range(NCORES):
        in_maps.append({
            "xT": np.ascontiguousarray(xT[m]),
            "wc0": wc0, "wc1": wc1, "b0r": b0r, "b1r": b1r,
            "iota": iota, "ident": ident,
            "srcg": np.ascontiguousarray(srcg[m]),
            "dstg": np.ascontiguousarray(dstg[m]),
            "dloc": np.ascontiguousarray(dloc[m]),
        })
    return in_maps, tuple(C_ws)


def _build(C_ws, stage=99):
    # stage for HW bisection: 1=phaseA+AG only, 2=+gathers, 3=+edge compute,
    # 4=+transposes (full layer0), 99=full 2 layers
    import concourse.bass as bass
    import concourse.bacc as bacc
    import concourse.mybir as mybir
    import concourse.tile as tile

    dt = mybir.dt
    alu = mybir.AluOpType
    act = mybir.ActivationFunctionType

    CT = int(np.sum(C_ws))
    GC8 = CT * 8
    CMAX = max(C_ws)

    nc = bacc.Bacc("TRN2", target_bir_lowering=False, debug=False,
                   num_devices=NCORES)

    xT_in = nc.dram_tensor("xT", [HD, BP], dt.bfloat16, kind="ExternalInput")
    wc0_in = nc.dram_tensor("wc0", [HD, 272], dt.bfloat16, kind="ExternalInput")
    wc1_in = nc.dram_tensor("wc1", [HD, 272], dt.bfloat16, kind="ExternalInput")
    b0r_in = nc.dram_tensor("b0r", [P, HD], dt.float32, kind="ExternalInput")
    b1r_in = nc.dram_tensor("b1r", [P, HD], dt.float32, kind="ExternalInput")
    iota_in = nc.dram_tensor("iota", [P, P], dt.float32, kind="ExternalInput")
    ident_in = nc.dram_tensor("ident", [P, P], dt.bfloat16, kind="ExternalInput")
    srcg_in = nc.dram_tensor("srcg", [P, GC8], dt.int16, kind="ExternalInput")
    dstg_in = nc.dram_tensor("dstg", [P, GC8], dt.int16, kind="ExternalInput")
    dloc_in = nc.dram_tensor("dloc", [P, CT], dt.float32, kind="ExternalInput")
    out_ext = nc.dram_tensor("out", [BP, HD], dt.float32, kind="ExternalOutput")

    with tile.TileContext(nc) as tc:
        with (
            tc.tile_pool(name="consts", bufs=1) as consts,
            tc.tile_pool(name="work", bufs=2) as work,
            tc.tile_pool(name="psum", bufs=2, space="PSUM") as psum,
            tc.tile_pool(name="dram", bufs=1, space="DRAM") as dram,
        ):
            # ---------- constants ----------
            wc_a = [consts.tile([P, 272], dt.bfloat16, name=f"wc_a{l}") for l in range(2)]
            wc_b = [consts.tile([P, 272], dt.bfloat16, name=f"wc_b{l}") for l in range(2)]
            for l, w_in in enumerate((wc0_in, wc1_in)):
                nc.sync.dma_start(out=wc_a[l][:], in_=w_in[0:P, :])
                nc.sync.dma_start(out=wc_b[l][:], in_=w_in[P:HD, :])
            brep = [consts.tile([P, HD], dt.float32, name=f"brep{l}") for l in range(2)]
            nc.sync.dma_start(out=brep[0][:], in_=b0r_in[:, :])
            nc.sync.dma_start(out=brep[1][:], in_=b1r_in[:, :])
            iota_t = consts.tile([P, P], dt.float32)
            nc.sync.dma_start(out=iota_t[:], in_=iota_in[:, :])
            ident_t = consts.tile([P, P], dt.bfloat16)
            nc.sync.dma_start(out=ident_t[:], in_=ident_in[:, :])
            x2T_a = consts.tile([P, BP], dt.bfloat16)
            x2T_b = consts.tile([P, BP], dt.bfloat16)

            hel_loc = [dram.tile([BP, TBL_W], dt.bfloat16, name=f"hel_loc{l}")
                       for l in range(2)]
            hel_all = [dram.tile([NB, TBL_W], dt.bfloat16, addr_space="Shared",
                                 name=f"hel_all{l}") for l in range(2)]

            for layer in range(1 if stage < 99 else 2):
                wa, wb = wc_a[layer], wc_b[layer]
                hloc, hall = hel_loc[layer], hel_all[layer]

                # ---------- Phase A: dense projections for own block ----------
                for t in range(NWIN):
                    cs = t * P
                    if layer == 0:
                        lhs_a = work.tile([P, P], dt.bfloat16, tag="lhs_a")
                        lhs_b = work.tile([P, P], dt.bfloat16, tag="lhs_b")
                        nc.sync.dma_start(out=lhs_a[:], in_=xT_in[0:P, cs:cs + P])
                        nc.sync.dma_start(out=lhs_b[:], in_=xT_in[P:HD, cs:cs + P])
                        la, lb = lhs_a[:], lhs_b[:]
                    else:
                        la, lb = x2T_a[:, cs:cs + P], x2T_b[:, cs:cs + P]
                    h_ps = psum.tile([P, 272], dt.float32, tag="h_ps")
                    nc.tensor.matmul(out=h_ps[:], lhsT=la, rhs=wa[:],
                                     start=True, stop=False)
                    nc.tensor.matmul(out=h_ps[:], lhsT=lb, rhs=wb[:],
                                     start=False, stop=True)
                    helrow = work.tile([P, TBL_W], dt.bfloat16, tag="helrow")
                    nc.vector.tensor_copy(out=helrow[:, 0:272], in_=h_ps[:])
                    nc.vector.memset(helrow[:, 272:TBL_W], 0.0)
                    nc.sync.dma_start(out=hloc[cs:cs + P, :], in_=helrow[:])

                nc.gpsimd.collective_compute(
                    "AllGather",
                    mybir.AluOpType.bypass,
                    replica_groups=[list(range(NCORES))],
                    ins=[hloc[:].opt()],
                    outs=[hall[:].opt()],
                )

                # ---------- Phase B: edge aggregation per dst window ----------
                if stage < 2:
                    continue
                _stage = 2 if stage in (21, 22, 23) else stage
                woff = 0
                woff8 = 0
                for w in range(NWIN):
                    Cw = C_ws[w]
                    L = Cw * P
                    base = w * P

                    srci = work.tile([P, CMAX * 8], dt.int16, tag="srci")
                    nc.sync.dma_start(out=srci[:, 0:Cw * 8],
                                      in_=srcg_in[:, woff8:woff8 + Cw * 8])
                    dsti = work.tile([P, CMAX * 8], dt.int16, tag="dsti")
                    nc.sync.dma_start(out=dsti[:, 0:Cw * 8],
                                      in_=dstg_in[:, woff8:woff8 + Cw * 8])
                    dloc_t = work.tile([P, CMAX], dt.float32, tag="dloc")
                    nc.sync.dma_start(out=dloc_t[:, 0:Cw],
                                      in_=dloc_in[:, woff:woff + Cw])

                    do_main = stage not in (23,) and not (stage == 21 and w > 0)
                    do_er = stage not in (22,) and not (stage == 21 and w > 0)
                    gath = work.tile([P, CMAX, TBL_W], dt.bfloat16, tag="gath")
                    if do_main:
                        nc.gpsimd.dma_gather(
                            out_ap=gath[:, 0:Cw, :],
                            in_ap=hall[:],
                            idxs_ap=srci[:, 0:Cw * 8],
                            num_idxs=L, num_idxs_reg=L,
                            elem_size=TBL_W, single_packet=False,
                        )
                    else:
                        nc.vector.memset(gath[:, 0:Cw, :], 0.0)
                    erg = work.tile([P, CMAX, P], dt.bfloat16, tag="erg")
                    if do_er:
                        nc.gpsimd.dma_gather(
                            out_ap=erg[:, 0:Cw, :],
                            in_ap=hloc[:, ELR_OFF:TBL_W],
                            idxs_ap=dsti[:, 0:Cw * 8],
                            num_idxs=L, num_idxs_reg=L,
                            elem_size=P, elem_step=TBL_W, single_packet=False,
                        )
                    else:
                        nc.vector.memset(erg[:, 0:Cw, :], 0.0)

                    if _stage < 3:
                        touch = work.tile([P, 64], dt.float32, tag="touch")
                        nc.vector.tensor_copy(out=touch[:], in_=gath[:, 0, 0:64])
                        nc.vector.tensor_tensor(out=touch[:], in0=touch[:],
                                                in1=erg[:, 0, 0:64], op=alu.add)
                        nc.sync.dma_start(out=out_ext[base:base + P, 0:64],
                                          in_=touch[:])
                        woff += Cw
                        woff8 += Cw * 8
                        continue

                    # ex = exp(leaky_relu(el_src + er_dst)) written into gath el slot
                    epre = work.tile([P, CMAX * 8], dt.float32, tag="epre")
                    ep = epre[:, 0:Cw * 8].rearrange("p (c h) -> p c h", h=8)
                    nc.vector.tensor_tensor(
                        out=ep, in0=gath[:, 0:Cw, 256:264],
                        in1=erg[:, 0:Cw, 8:16], op=alu.add)
                    # leaky_relu(x) = max(x, NEG*x)
                    escl = work.tile([P, CMAX * 8], dt.float32, tag="escl")
                    es = escl[:, 0:Cw * 8].rearrange("p (c h) -> p c h", h=8)
                    nc.vector.tensor_scalar(out=es, in0=ep, scalar1=NEG,
                                            scalar2=None, op0=alu.mult)
                    nc.vector.tensor_tensor(out=ep, in0=ep, in1=es, op=alu.max)
                    nc.scalar.activation(out=gath[:, 0:Cw, 256:264], in_=ep,
                                         func=act.Exp)
                    # h *= ex (per-head broadcast over 32 dims)
                    feat4 = gath[:, 0:Cw, 0:HD].rearrange("p c (h d) -> p c h d", d=D)
                    exb = gath[:, 0:Cw, 256:264].unsqueeze(3).to_broadcast(
                        [P, Cw, 8, D])
                    nc.vector.tensor_tensor(out=feat4, in0=feat4, in1=exb,
                                            op=alu.mult)

                    agg = psum.tile([P, 264], dt.float32, tag="agg")
                    for c in range(Cw):
                        oh = work.tile([P, P], dt.bfloat16, tag="oh", bufs=3)
                        nc.vector.tensor_scalar(
                            out=oh[:], in0=iota_t[:],
                            scalar1=dloc_t[:, c:c + 1], scalar2=None,
                            op0=alu.is_equal)
                        nc.tensor.matmul(
                            out=agg[:], lhsT=oh[:], rhs=gath[:, c, 0:264],
                            start=(c == 0), stop=(c == Cw - 1))

                    # normalize + bias + ELU
                    sinv = work.tile([P, 8], dt.float32, tag="sinv")
                    nc.vector.tensor_scalar(out=sinv[:], in0=agg[:, 256:264],
                                            scalar1=1e-12, scalar2=None,
                                            op0=alu.max)
                    nc.vector.reciprocal(out=sinv[:], in_=sinv[:])
                    xw = work.tile([P, HD], dt.float32, tag="xw")
                    xw4 = xw[:].rearrange("p (h d) -> p h d", d=D)
                    nc.vector.tensor_tensor(
                        out=xw4,
                        in0=agg[:, 0:HD].rearrange("p (h d) -> p h d", d=D),
                        in1=sinv[:].unsqueeze(2).to_broadcast([P, 8, D]),
                        op=alu.mult)
                    nc.vector.tensor_tensor(out=xw[:], in0=xw[:],
                                            in1=brep[layer][:], op=alu.add)
                    # elu(x) = (max(x,0) - 1) + exp(min(x,0))
                    emin = work.tile([P, HD], dt.float32, tag="emin")
                    nc.vector.tensor_scalar(out=emin[:], in0=xw[:], scalar1=0.0,
                                            scalar2=None, op0=alu.min)
                    nc.scalar.activation(out=emin[:], in_=emin[:], func=act.Exp)
                    nc.vector.tensor_scalar(out=xw[:], in0=xw[:], scalar1=0.0,
                                            scalar2=-1.0, op0=alu.max,
                                            op1=alu.add)
                    nc.vector.tensor_tensor(out=xw[:], in0=xw[:], in1=emin[:],
                                            op=alu.add)

                    if layer == 0 and stage == 3:
                        nc.sync.dma_start(out=out_ext[base:base + P, :],
                                          in_=xw[:])
                    elif layer == 0:
                        xwb = work.tile([P, HD], dt.bfloat16, tag="xwb")
                        nc.vector.tensor_copy(out=xwb[:], in_=xw[:])
                        for half, x2T in ((0, x2T_a), (1, x2T_b)):
                            tp = psum.tile([P, P], dt.bfloat16, tag="tp")
                            nc.tensor.transpose(
                                out=tp[:], in_=xwb[:, half * P:(half + 1) * P],
                                identity=ident_t[:])
                            nc.vector.tensor_copy(out=x2T[:, base:base + P],
                                                  in_=tp[:])
                    else:
                        nc.sync.dma_start(out=out_ext[base:base + P, :],
                                          in_=xw[:])
                    woff += Cw
                    woff8 += Cw * 8

    nc.compile()
    return nc


def _run(inputs, trace=False):
    from concourse.bass_utils import run_bass_kernel_spmd

    in_maps, C_ws = _host_prep(**inputs)
    key = C_ws
    if key not in _BUILD_CACHE:
        _BUILD_CACHE[key] = _build(C_ws)
    nc = _BUILD_CACHE[key]
    r = run_bass_kernel_spmd(nc, in_maps, core_ids=list(range(NCORES)),
                             trace=trace)
    out = np.empty((N, HD), np.float32)
    for m in range(NCORES):
        out[m * BLK:(m + 1) * BLK] = r.results[m]["out"][:BLK]
    return out, r.exec_time_ns


def kernel(**inputs):
    inputs = {k: np.asarray(v) for k, v in inputs.items()}
    out, _ = _run(inputs, trace=False)
    return out


# revision 12
# speedup vs baseline: 1.2612x; 1.2612x over previous
"""2-layer GAT (DGL GATConv eval-mode) on 8 Trainium2 NeuronCores.

Strategy:
  - Nodes are partitioned into 8 blocks of 2500 (padded to 2560 = 20 windows
    of 128). Core m owns block m: it computes the dense projections for its
    own nodes and aggregates messages for destination nodes in its block.
  - Per layer:
    Phase A (dense, sharded): h|el|er = x_blk @ [W | W@al | W@ar] via
      TensorE (bf16), written as a 384-col bf16 node table
      [h(256) | el(8) | er(8) | pad(112)] -> 768B rows; AllGather -> full
      table on every core.
    Phase B (edges, dst-sharded): edges sorted by dst, grouped into
      128-dst windows, chunks of 128 edges. Per window: one dma_gather of
      full 768B rows by src (gives h and el per edge), one 256B dma_gather
      of [el|er|pad] by local dst (gives er per edge; reads the local
      pre-AG table). ex = exp(leaky_relu(el_src + er_dst)). One-hot
      (iota == dstloc) matmuls accumulate both sum(ex*h) and sum(ex) into
      PSUM in a single 264-wide matmul per 128-edge chunk. Normalize by
      1/sum(ex), add bias, ELU.
  - Layer-1 outputs are PE-transposed into an SBUF-resident x2T so layer 2's
    dense phase needs no DMA for activations.
"""
import sys

for _p in ("/opt/trn_rl_repo",):
    if _p not in sys.path:
        sys.path.append(_p)

import numpy as np

N = 20000
E = 320000
H = 8
D = 32
HD = H * D          # 256
NEG = 0.2
P = 128
NCORES = 8
BLK = 2500          # real nodes per core block
NWIN = 20           # windows per core
BP = NWIN * P       # padded block stride 2560
NB = NCORES * BP    # padded global rows 20480
TBL_W = 384         # table row: h 0:256 | el 256:264 | er 264:272 | pad
ELR_OFF = 256       # er-gather reads cols [256:384]; er at relative 8:16

_BUILD_CACHE = {}


def _host_prep(features, W0, al0, ar0, b0, W1, al1, ar1, b1, src, dst):
    import ml_dtypes
    bf16 = ml_dtypes.bfloat16

    def wcat(W, al, ar):
        Wal = np.stack([W[:, h * D:(h + 1) * D] @ al[h] for h in range(H)], axis=1)
        War = np.stack([W[:, h * D:(h + 1) * D] @ ar[h] for h in range(H)], axis=1)
        return np.ascontiguousarray(
            np.concatenate([W, Wal, War], axis=1).astype(bf16))  # [256, 272]

    wc0 = wcat(W0, al0, ar0)
    wc1 = wcat(W1, al1, ar1)
    b0r = np.ascontiguousarray(np.tile(b0[None, :], (P, 1)).astype(np.float32))
    b1r = np.ascontiguousarray(np.tile(b1[None, :], (P, 1)).astype(np.float32))
    iota = np.ascontiguousarray(
        np.tile(np.arange(P, dtype=np.float32)[None, :], (P, 1)))
    ident = np.eye(P, dtype=np.float32).astype(bf16)

    # per-core transposed, block-padded features [256, BP]
    xT = np.zeros((NCORES, HD, BP), dtype=bf16)
    xf = features.astype(bf16)
    for m in range(NCORES):
        xT[m, :, :BLK] = xf[m * BLK:(m + 1) * BLK].T

    # ---- edge partitioning ----
    owner = dst // BLK
    dst_local = dst - owner * BLK          # [0, 2500)
    win = dst_local // P                   # [0, 20)
    src_blocked = (src // BLK) * BP + (src % BLK)

    # per (core, window) edge lists
    counts = np.zeros((NCORES, NWIN), np.int64)
    lists = [[None] * NWIN for _ in range(NCORES)]
    for m in range(NCORES):
        sel = np.nonzero(owner == m)[0]
        w_m = win[sel]
        order = np.argsort(w_m, kind="stable")
        sel = sel[order]
        w_m = w_m[order]
        bounds = np.searchsorted(w_m, np.arange(NWIN + 1))
        for w in range(NWIN):
            ee = sel[bounds[w]:bounds[w + 1]]
            lists[m][w] = ee
            counts[m, w] = len(ee)

    C_ws = [int(np.ceil(counts[:, w].max() / P)) for w in range(NWIN)]
    CT = int(np.sum(C_ws))
    GC8 = CT * 8

    srcg = np.zeros((NCORES, P, GC8), np.int16)
    dstg = np.zeros((NCORES, P, GC8), np.int16)
    dloc = np.full((NCORES, P, CT), -1.0, np.float32)

    for m in range(NCORES):
        woff = 0
        woff8 = 0
        for w in range(NWIN):
            Cw = C_ws[w]
            L = Cw * P
            ee = lists[m][w]
            sb = np.zeros(L, np.int16)
            db = np.zeros(L, np.int16)
            dl = np.full(L, -1.0, np.float32)
            n = len(ee)
            sb[:n] = src_blocked[ee]
            db[:n] = dst_local[ee]
            dl[:n] = (dst_local[ee] - w * P).astype(np.float32)
            # gather idx layout: index i at [i%16, i//16], replicated x8 down
            sg = sb.reshape(Cw * 8, 16).T      # [16, Cw*8]
            dg = db.reshape(Cw * 8, 16).T
            srcg[m, :, woff8:woff8 + Cw * 8] = np.tile(sg, (8, 1))
            dstg[m, :, woff8:woff8 + Cw * 8] = np.tile(dg, (8, 1))
            # dstloc layout: edge i at [i%128, i//128]
            dloc[m, :, woff:woff + Cw] = dl.reshape(Cw, P).T
            woff += Cw
            woff8 += Cw * 8

    in_maps = []
    for m in range(NCORES):
        in_maps.append({
            "xT": np.ascontiguousarray(xT[m]),
            "wc0": wc0, "wc1": wc1, "b0r": b0r, "b1r": b1r,
            "iota": iota, "ident": ident,
            "srcg": np.ascontiguousarray(srcg[m]),
            "dstg": np.ascontiguousarray(dstg[m]),
            "dloc": np.ascontiguousarray(dloc[m]),
        })
    return in_maps, tuple(C_ws)


def _build(C_ws, stage=99):
    # stage for HW bisection: 1=phaseA+AG only, 2=+gathers, 3=+edge compute,
    # 4=+transposes (full layer0), 99=full 2 layers
    import concourse.bass as bass
    import concourse.bacc as bacc
    import concourse.mybir as mybir
    import concourse.tile as tile

    dt = mybir.dt
    alu = mybir.AluOpType
    act = mybir.ActivationFunctionType

    CT = int(np.sum(C_ws))
    GC8 = CT * 8
    CMAX = max(C_ws)

    nc = bacc.Bacc("TRN2", target_bir_lowering=False, debug=False,
                   num_devices=NCORES, num_swdge_queues=4)

    xT_in = nc.dram_tensor("xT", [HD, BP], dt.bfloat16, kind="ExternalInput")
    wc0_in = nc.dram_tensor("wc0", [HD, 272], dt.bfloat16, kind="ExternalInput")
    wc1_in = nc.dram_tensor("wc1", [HD, 272], dt.bfloat16, kind="ExternalInput")
    b0r_in = nc.dram_tensor("b0r", [P, HD], dt.float32, kind="ExternalInput")
    b1r_in = nc.dram_tensor("b1r", [P, HD], dt.float32, kind="ExternalInput")
    iota_in = nc.dram_tensor("iota", [P, P], dt.float32, kind="ExternalInput")
    ident_in = nc.dram_tensor("ident", [P, P], dt.bfloat16, kind="ExternalInput")
    srcg_in = nc.dram_tensor("srcg", [P, GC8], dt.int16, kind="ExternalInput")
    dstg_in = nc.dram_tensor("dstg", [P, GC8], dt.int16, kind="ExternalInput")
    dloc_in = nc.dram_tensor("dloc", [P, CT], dt.float32, kind="ExternalInput")
    out_ext = nc.dram_tensor("out", [BP, HD], dt.float32, kind="ExternalOutput")

    with tile.TileContext(nc) as tc:
        with (
            tc.tile_pool(name="consts", bufs=1) as consts,
            tc.tile_pool(name="work", bufs=2) as work,
            tc.tile_pool(name="psum", bufs=2, space="PSUM") as psum,
            tc.tile_pool(name="dram", bufs=1, space="DRAM") as dram,
        ):
            # ---------- constants ----------
            wc_a = [consts.tile([P, 272], dt.bfloat16, name=f"wc_a{l}") for l in range(2)]
            wc_b = [consts.tile([P, 272], dt.bfloat16, name=f"wc_b{l}") for l in range(2)]
            for l, w_in in enumerate((wc0_in, wc1_in)):
                nc.sync.dma_start(out=wc_a[l][:], in_=w_in[0:P, :])
                nc.sync.dma_start(out=wc_b[l][:], in_=w_in[P:HD, :])
            brep = [consts.tile([P, HD], dt.float32, name=f"brep{l}") for l in range(2)]
            nc.sync.dma_start(out=brep[0][:], in_=b0r_in[:, :])
            nc.sync.dma_start(out=brep[1][:], in_=b1r_in[:, :])
            iota_t = consts.tile([P, P], dt.float32)
            nc.sync.dma_start(out=iota_t[:], in_=iota_in[:, :])
            ident_t = consts.tile([P, P], dt.bfloat16)
            nc.sync.dma_start(out=ident_t[:], in_=ident_in[:, :])
            x2T_a = consts.tile([P, BP], dt.bfloat16)
            x2T_b = consts.tile([P, BP], dt.bfloat16)

            hel_loc = [dram.tile([BP, TBL_W], dt.bfloat16, name=f"hel_loc{l}")
                       for l in range(2)]
            hel_all = [dram.tile([NB, TBL_W], dt.bfloat16, addr_space="Shared",
                                 name=f"hel_all{l}") for l in range(2)]

            for layer in range(1 if stage < 99 else 2):
                wa, wb = wc_a[layer], wc_b[layer]
                hloc, hall = hel_loc[layer], hel_all[layer]

                # ---------- Phase A: dense projections for own block ----------
                for t in range(NWIN):
                    cs = t * P
                    if layer == 0:
                        lhs_a = work.tile([P, P], dt.bfloat16, tag="lhs_a")
                        lhs_b = work.tile([P, P], dt.bfloat16, tag="lhs_b")
                        nc.sync.dma_start(out=lhs_a[:], in_=xT_in[0:P, cs:cs + P])
                        nc.sync.dma_start(out=lhs_b[:], in_=xT_in[P:HD, cs:cs + P])
                        la, lb = lhs_a[:], lhs_b[:]
                    else:
                        la, lb = x2T_a[:, cs:cs + P], x2T_b[:, cs:cs + P]
                    h_ps = psum.tile([P, 272], dt.float32, tag="h_ps")
                    nc.tensor.matmul(out=h_ps[:], lhsT=la, rhs=wa[:],
                                     start=True, stop=False)
                    nc.tensor.matmul(out=h_ps[:], lhsT=lb, rhs=wb[:],
                                     start=False, stop=True)
                    helrow = work.tile([P, TBL_W], dt.bfloat16, tag="helrow")
                    nc.scalar.copy(out=helrow[:, 0:272], in_=h_ps[:])
                    nc.vector.memset(helrow[:, 272:TBL_W], 0.0)
                    nc.sync.dma_start(out=hloc[cs:cs + P, :], in_=helrow[:])

                nc.gpsimd.collective_compute(
                    "AllGather",
                    mybir.AluOpType.bypass,
                    replica_groups=[list(range(NCORES))],
                    ins=[hloc[:].opt()],
                    outs=[hall[:].opt()],
                )

                # ---------- Phase B: edge aggregation per dst window ----------
                if stage < 2:
                    continue
                _stage = 2 if stage in (21, 22, 23) else stage
                woff = 0
                woff8 = 0
                for w in range(NWIN):
                    Cw = C_ws[w]
                    L = Cw * P
                    base = w * P

                    srci = work.tile([P, CMAX * 8], dt.int16, tag="srci")
                    nc.sync.dma_start(out=srci[:, 0:Cw * 8],
                                      in_=srcg_in[:, woff8:woff8 + Cw * 8])
                    dsti = work.tile([P, CMAX * 8], dt.int16, tag="dsti")
                    nc.sync.dma_start(out=dsti[:, 0:Cw * 8],
                                      in_=dstg_in[:, woff8:woff8 + Cw * 8])
                    dloc_t = work.tile([P, CMAX], dt.float32, tag="dloc")
                    nc.sync.dma_start(out=dloc_t[:, 0:Cw],
                                      in_=dloc_in[:, woff:woff + Cw])

                    do_main = stage not in (23,) and not (stage == 21 and w > 0)
                    do_er = stage not in (22,) and not (stage == 21 and w > 0)
                    gath = work.tile([P, CMAX, TBL_W], dt.bfloat16, tag="gath")
                    if do_main:
                        nc.gpsimd.dma_gather(
                            out_ap=gath[:, 0:Cw, :],
                            in_ap=hall[:],
                            idxs_ap=srci[:, 0:Cw * 8],
                            num_idxs=L, num_idxs_reg=L,
                            elem_size=TBL_W, single_packet=False,
                            queue_num=(2 * w) % 4,
                        )
                    else:
                        nc.vector.memset(gath[:, 0:Cw, :], 0.0)
                    erg = work.tile([P, CMAX, P], dt.bfloat16, tag="erg")
                    if do_er:
                        nc.gpsimd.dma_gather(
                            out_ap=erg[:, 0:Cw, :],
                            in_ap=hloc[:, ELR_OFF:TBL_W],
                            idxs_ap=dsti[:, 0:Cw * 8],
                            num_idxs=L, num_idxs_reg=L,
                            elem_size=P, elem_step=TBL_W, single_packet=False,
                            queue_num=(2 * w + 1) % 4,
                        )
                    else:
                        nc.vector.memset(erg[:, 0:Cw, :], 0.0)

                    if _stage < 3:
                        touch = work.tile([P, 64], dt.float32, tag="touch")
                        nc.vector.tensor_copy(out=touch[:], in_=gath[:, 0, 0:64])
                        nc.vector.tensor_tensor(out=touch[:], in0=touch[:],
                                                in1=erg[:, 0, 0:64], op=alu.add)
                        nc.sync.dma_start(out=out_ext[base:base + P, 0:64],
                                          in_=touch[:])
                        woff += Cw
                        woff8 += Cw * 8
                        continue

                    # ex = exp(leaky_relu(el_src + er_dst)) written into gath el slot
                    epre = work.tile([P, CMAX * 8], dt.float32, tag="epre")
                    ep = epre[:, 0:Cw * 8].rearrange("p (c h) -> p c h", h=8)
                    nc.vector.tensor_tensor(
                        out=ep, in0=gath[:, 0:Cw, 256:264],
                        in1=erg[:, 0:Cw, 8:16], op=alu.add)
                    # leaky_relu(x) = max(x, NEG*x)
                    escl = work.tile([P, CMAX * 8], dt.float32, tag="escl")
                    es = escl[:, 0:Cw * 8].rearrange("p (c h) -> p c h", h=8)
                    nc.vector.tensor_scalar(out=es, in0=ep, scalar1=NEG,
                                            scalar2=None, op0=alu.mult)
                    nc.vector.tensor_tensor(out=ep, in0=ep, in1=es, op=alu.max)
                    nc.scalar.activation(out=gath[:, 0:Cw, 256:264], in_=ep,
                                         func=act.Exp)
                    # h *= ex (per-head broadcast over 32 dims)
                    feat4 = gath[:, 0:Cw, 0:HD].rearrange("p c (h d) -> p c h d", d=D)
                    exb = gath[:, 0:Cw, 256:264].unsqueeze(3).to_broadcast(
                        [P, Cw, 8, D])
                    nc.vector.tensor_tensor(out=feat4, in0=feat4, in1=exb,
                                            op=alu.mult)

                    agg = psum.tile([P, 264], dt.float32, tag="agg")
                    oh3 = work.tile([P, CMAX, P], dt.bfloat16, tag="oh")
                    nc.vector.tensor_tensor(
                        out=oh3[:, 0:Cw, :],
                        in0=dloc_t[:, 0:Cw].unsqueeze(2).to_broadcast([P, Cw, P]),
                        in1=iota_t[:].unsqueeze(1).to_broadcast([P, Cw, P]),
                        op=alu.is_equal)
                    for c in range(Cw):
                        nc.tensor.matmul(
                            out=agg[:], lhsT=oh3[:, c, :], rhs=gath[:, c, 0:264],
                            start=(c == 0), stop=(c == Cw - 1))

                    # normalize + bias + ELU
                    sinv = work.tile([P, 8], dt.float32, tag="sinv")
                    nc.vector.tensor_scalar(out=sinv[:], in0=agg[:, 256:264],
                                            scalar1=1e-12, scalar2=None,
                                            op0=alu.max)
                    nc.vector.reciprocal(out=sinv[:], in_=sinv[:])
                    xw = work.tile([P, HD], dt.float32, tag="xw")
                    xw4 = xw[:].rearrange("p (h d) -> p h d", d=D)
                    nc.vector.tensor_tensor(
                        out=xw4,
                        in0=agg[:, 0:HD].rearrange("p (h d) -> p h d", d=D),
                        in1=sinv[:].unsqueeze(2).to_broadcast([P, 8, D]),
                        op=alu.mult)
                    nc.vector.tensor_tensor(out=xw[:], in0=xw[:],
                                            in1=brep[layer][:], op=alu.add)
                    # elu(x) = (max(x,0) - 1) + exp(min(x,0))
                    emin = work.tile([P, HD], dt.float32, tag="emin")
                    nc.vector.tensor_scalar(out=emin[:], in0=xw[:], scalar1=0.0,
                                            scalar2=None, op0=alu.min)
                    nc.scalar.activation(out=emin[:], in_=emin[:], func=act.Exp)
                    nc.vector.tensor_scalar(out=xw[:], in0=xw[:], scalar1=0.0,
                                            scalar2=-1.0, op0=alu.max,
                                            op1=alu.add)
                    nc.vector.tensor_tensor(out=xw[:], in0=xw[:], in1=emin[:],
                                            op=alu.add)

                    if layer == 0 and stage == 3:
                        nc.sync.dma_start(out=out_ext[base:base + P, :],
                                          in_=xw[:])
                    elif layer == 0:
                        xwb = work.tile([P, HD], dt.bfloat16, tag="xwb")
                        nc.scalar.copy(out=xwb[:], in_=xw[:])
                        for half, x2T in ((0, x2T_a), (1, x2T_b)):
                            tp = psum.tile([P, P], dt.bfloat16, tag="tp")
                            nc.tensor.transpose(
                                out=tp[:], in_=xwb[:, half * P:(half + 1) * P],
                                identity=ident_t[:])
                            nc.scalar.copy(out=x2T[:, base:base + P],
                                           in_=tp[:])
                    else:
                        nc.sync.dma_start(out=out_ext[base:base + P, :],
                                          in_=xw[:])
                    woff += Cw
                    woff8 += Cw * 8

    nc.compile()
    return nc


def _run(inputs, trace=False):
    from concourse.bass_utils import run_bass_kernel_spmd

    in_maps, C_ws = _host_prep(**inputs)
    key = C_ws
    if key not in _BUILD_CACHE:
        _BUILD_CACHE[key] = _build(C_ws)
    nc = _BUILD_CACHE[key]
    r = run_bass_kernel_spmd(nc, in_maps, core_ids=list(range(NCORES)),
                             trace=trace)
    out = np.empty((N, HD), np.float32)
    for m in range(NCORES):
        out[m * BLK:(m + 1) * BLK] = r.results[m]["out"][:BLK]
    return out, r.exec_time_ns


def kernel(**inputs):
    inputs = {k: np.asarray(v) for k, v in inputs.items()}
    out, _ = _run(inputs, trace=False)
    return out


# revision 14
# speedup vs baseline: 1.5908x; 1.2613x over previous
"""2-layer GAT (DGL GATConv eval-mode) on 8 Trainium2 NeuronCores.

Strategy:
  - Nodes are partitioned into 8 blocks of 2500 (padded to 2560 = 20 windows
    of 128). Core m owns block m: it computes the dense projections for its
    own nodes and aggregates messages for destination nodes in its block.
  - Per layer:
    Phase A (dense, sharded): h|el|er = x_blk @ [W | W@al | W@ar] via
      TensorE (bf16), written as a 384-col bf16 node table
      [h(256) | el(8) | er(8) | pad(112)] -> 768B rows; AllGather -> full
      table on every core.
    Phase B (edges, dst-sharded): edges sorted by dst, grouped into
      128-dst windows, chunks of 128 edges. Per window: one dma_gather of
      full 768B rows by src (gives h and el per edge), one 256B dma_gather
      of [el|er|pad] by local dst (gives er per edge; reads the local
      pre-AG table). ex = exp(leaky_relu(el_src + er_dst)). One-hot
      (iota == dstloc) matmuls accumulate both sum(ex*h) and sum(ex) into
      PSUM in a single 264-wide matmul per 128-edge chunk. Normalize by
      1/sum(ex), add bias, ELU.
  - Layer-1 outputs are PE-transposed into an SBUF-resident x2T so layer 2's
    dense phase needs no DMA for activations.
"""
import sys

for _p in ("/opt/trn_rl_repo",):
    if _p not in sys.path:
        sys.path.append(_p)

import numpy as np

N = 20000
E = 320000
H = 8
D = 32
HD = H * D          # 256
NEG = 0.2
P = 128
NCORES = 8
BLK = 2500          # real nodes per core block
NWIN = 20           # windows per core
BP = NWIN * P       # padded block stride 2560
NB = NCORES * BP    # padded global rows 20480
TBL_W = 384         # table row: h 0:256 | el 256:264 | er 264:272 | pad
ELR_OFF = 256       # er-gather reads cols [256:384]; er at relative 8:16

_BUILD_CACHE = {}


def _host_prep(features, W0, al0, ar0, b0, W1, al1, ar1, b1, src, dst):
    import ml_dtypes
    bf16 = ml_dtypes.bfloat16

    def wcat(W, al, ar):
        Wal = np.stack([W[:, h * D:(h + 1) * D] @ al[h] for h in range(H)], axis=1)
        War = np.stack([W[:, h * D:(h + 1) * D] @ ar[h] for h in range(H)], axis=1)
        return np.ascontiguousarray(
            np.concatenate([W, Wal, War], axis=1).astype(bf16))  # [256, 272]

    wc0 = wcat(W0, al0, ar0)
    wc1 = wcat(W1, al1, ar1)
    b0r = np.ascontiguousarray(np.tile(b0[None, :], (P, 1)).astype(np.float32))
    b1r = np.ascontiguousarray(np.tile(b1[None, :], (P, 1)).astype(np.float32))
    iota = np.ascontiguousarray(
        np.tile(np.arange(P, dtype=np.float32)[None, :], (P, 1)).astype(bf16))
    ident = np.eye(P, dtype=np.float32).astype(bf16)

    # per-core transposed, block-padded features [256, BP]
    xT = np.zeros((NCORES, HD, BP), dtype=bf16)
    xf = features.astype(bf16)
    for m in range(NCORES):
        xT[m, :, :BLK] = xf[m * BLK:(m + 1) * BLK].T

    # ---- edge partitioning ----
    owner = dst // BLK
    dst_local = dst - owner * BLK          # [0, 2500)
    win = dst_local // P                   # [0, 20)
    src_blocked = (src // BLK) * BP + (src % BLK)

    # per (core, window) edge lists
    counts = np.zeros((NCORES, NWIN), np.int64)
    lists = [[None] * NWIN for _ in range(NCORES)]
    for m in range(NCORES):
        sel = np.nonzero(owner == m)[0]
        w_m = win[sel]
        order = np.argsort(w_m, kind="stable")
        sel = sel[order]
        w_m = w_m[order]
        bounds = np.searchsorted(w_m, np.arange(NWIN + 1))
        for w in range(NWIN):
            ee = sel[bounds[w]:bounds[w + 1]]
            lists[m][w] = ee
            counts[m, w] = len(ee)

    C_ws = [int(np.ceil(counts[:, w].max() / P)) for w in range(NWIN)]
    CT = int(np.sum(C_ws))
    GC8 = CT * 8

    srcg = np.zeros((NCORES, P, GC8), np.int16)
    dloc = np.full((NCORES, P, CT), -1.0, np.float32)
    ohT = np.zeros((NCORES, P, CT * P), bf16)

    for m in range(NCORES):
        woff = 0
        woff8 = 0
        for w in range(NWIN):
            Cw = C_ws[w]
            L = Cw * P
            ee = lists[m][w]
            sb = np.zeros(L, np.int16)
            dl = np.full(L, -1.0, np.float32)
            n = len(ee)
            sb[:n] = src_blocked[ee]
            dl[:n] = (dst_local[ee] - w * P).astype(np.float32)
            # gather idx layout: index i at [i%16, i//16], replicated x8 down
            sg = sb.reshape(Cw * 8, 16).T      # [16, Cw*8]
            srcg[m, :, woff8:woff8 + Cw * 8] = np.tile(sg, (8, 1))
            # dstloc layout: edge i at [i%128, i//128]
            dloc[m, :, woff:woff + Cw] = dl.reshape(Cw, P).T
            # transposed one-hot: ohT[d, (woff+c)*128 + j] = (dstloc(edge c*128+j) == d)
            valid = np.nonzero(dl >= 0)[0]
            ohT[m, dl[valid].astype(np.int64), woff * P + valid] = 1.0
            woff += Cw
            woff8 += Cw * 8

    in_maps = []
    for m in range(NCORES):
        in_maps.append({
            "xT": np.ascontiguousarray(xT[m]),
            "wc0": wc0, "wc1": wc1, "b0r": b0r, "b1r": b1r,
            "iota": iota, "ident": ident,
            "srcg": np.ascontiguousarray(srcg[m]),
            "ohT": np.ascontiguousarray(ohT[m]),
            "dloc": np.ascontiguousarray(dloc[m].astype(bf16)),
        })
    return in_maps, tuple(C_ws)


def _build(C_ws, stage=99):
    # stage for HW bisection: 1=phaseA+AG only, 2=+gathers, 3=+edge compute,
    # 4=+transposes (full layer0), 99=full 2 layers
    import concourse.bass as bass
    import concourse.bacc as bacc
    import concourse.mybir as mybir
    import concourse.tile as tile

    dt = mybir.dt
    alu = mybir.AluOpType
    act = mybir.ActivationFunctionType

    CT = int(np.sum(C_ws))
    GC8 = CT * 8
    CMAX = max(C_ws)

    nc = bacc.Bacc("TRN2", target_bir_lowering=False, debug=False,
                   num_devices=NCORES, num_swdge_queues=4)

    xT_in = nc.dram_tensor("xT", [HD, BP], dt.bfloat16, kind="ExternalInput")
    wc0_in = nc.dram_tensor("wc0", [HD, 272], dt.bfloat16, kind="ExternalInput")
    wc1_in = nc.dram_tensor("wc1", [HD, 272], dt.bfloat16, kind="ExternalInput")
    b0r_in = nc.dram_tensor("b0r", [P, HD], dt.float32, kind="ExternalInput")
    b1r_in = nc.dram_tensor("b1r", [P, HD], dt.float32, kind="ExternalInput")
    iota_in = nc.dram_tensor("iota", [P, P], dt.bfloat16, kind="ExternalInput")
    ident_in = nc.dram_tensor("ident", [P, P], dt.bfloat16, kind="ExternalInput")
    srcg_in = nc.dram_tensor("srcg", [P, GC8], dt.int16, kind="ExternalInput")
    ohT_in = nc.dram_tensor("ohT", [P, CT * P], dt.bfloat16, kind="ExternalInput")
    dloc_in = nc.dram_tensor("dloc", [P, CT], dt.bfloat16, kind="ExternalInput")
    out_ext = nc.dram_tensor("out", [BP, HD], dt.float32, kind="ExternalOutput")

    with tile.TileContext(nc) as tc:
        with (
            tc.tile_pool(name="consts", bufs=1) as consts,
            tc.tile_pool(name="work", bufs=2) as work,
            tc.tile_pool(name="psum", bufs=2, space="PSUM") as psum,
            tc.tile_pool(name="dram", bufs=1, space="DRAM") as dram,
        ):
            # ---------- constants ----------
            wc_a = [consts.tile([P, 272], dt.bfloat16, name=f"wc_a{l}") for l in range(2)]
            wc_b = [consts.tile([P, 272], dt.bfloat16, name=f"wc_b{l}") for l in range(2)]
            for l, w_in in enumerate((wc0_in, wc1_in)):
                nc.sync.dma_start(out=wc_a[l][:], in_=w_in[0:P, :])
                nc.sync.dma_start(out=wc_b[l][:], in_=w_in[P:HD, :])
            brep = [consts.tile([P, HD], dt.float32, name=f"brep{l}") for l in range(2)]
            nc.sync.dma_start(out=brep[0][:], in_=b0r_in[:, :])
            nc.sync.dma_start(out=brep[1][:], in_=b1r_in[:, :])
            iota_t = consts.tile([P, P], dt.bfloat16)
            nc.sync.dma_start(out=iota_t[:], in_=iota_in[:, :])
            ident_t = consts.tile([P, P], dt.bfloat16)
            nc.sync.dma_start(out=ident_t[:], in_=ident_in[:, :])
            x2T_a = consts.tile([P, BP], dt.bfloat16)
            x2T_b = consts.tile([P, BP], dt.bfloat16)
            eps_t = consts.tile([P, 8], dt.float32)
            nc.vector.memset(eps_t[:], 1e-12)
            zero_t = consts.tile([P, HD], dt.float32)
            nc.vector.memset(zero_t[:], 0.0)
            mone_t = consts.tile([P, HD], dt.float32)
            nc.vector.memset(mone_t[:], -1.0)
            er_own = [consts.tile([P, NWIN * 8], dt.bfloat16, name=f"er_own{l}")
                      for l in range(2)]

            hel_loc = [dram.tile([BP, TBL_W], dt.bfloat16, name=f"hel_loc{l}")
                       for l in range(2)]
            hel_all = [dram.tile([NB, TBL_W], dt.bfloat16, addr_space="Shared",
                                 name=f"hel_all{l}") for l in range(2)]

            for layer in range(1 if stage < 99 else 2):
                wa, wb = wc_a[layer], wc_b[layer]
                hloc, hall = hel_loc[layer], hel_all[layer]

                # ---------- Phase A: dense projections for own block ----------
                for t in range(NWIN):
                    cs = t * P
                    if layer == 0:
                        lhs_a = work.tile([P, P], dt.bfloat16, tag="lhs_a")
                        lhs_b = work.tile([P, P], dt.bfloat16, tag="lhs_b")
                        nc.sync.dma_start(out=lhs_a[:], in_=xT_in[0:P, cs:cs + P])
                        nc.sync.dma_start(out=lhs_b[:], in_=xT_in[P:HD, cs:cs + P])
                        la, lb = lhs_a[:], lhs_b[:]
                    else:
                        la, lb = x2T_a[:, cs:cs + P], x2T_b[:, cs:cs + P]
                    h_ps = psum.tile([P, 272], dt.float32, tag="h_ps")
                    nc.tensor.matmul(out=h_ps[:], lhsT=la, rhs=wa[:],
                                     start=True, stop=False)
                    nc.tensor.matmul(out=h_ps[:], lhsT=lb, rhs=wb[:],
                                     start=False, stop=True)
                    helrow = work.tile([P, TBL_W], dt.bfloat16, tag="helrow")
                    nc.scalar.copy(out=helrow[:, 0:272], in_=h_ps[:])
                    nc.scalar.copy(out=er_own[layer][:, t * 8:(t + 1) * 8],
                                   in_=h_ps[:, 264:272])
                    nc.vector.memset(helrow[:, 272:TBL_W], 0.0)
                    nc.sync.dma_start(out=hloc[cs:cs + P, :], in_=helrow[:])

                nc.gpsimd.collective_compute(
                    "AllGather",
                    mybir.AluOpType.bypass,
                    replica_groups=[list(range(NCORES))],
                    ins=[hloc[:].opt()],
                    outs=[hall[:].opt()],
                )

                # ---------- Phase B: edge aggregation per dst window ----------
                if stage < 2:
                    continue
                _stage = 2 if stage in (21, 22, 23) else stage
                woff = 0
                woff8 = 0
                for w in range(NWIN):
                    Cw = C_ws[w]
                    L = Cw * P
                    base = w * P

                    srci = work.tile([P, CMAX * 8], dt.int16, tag="srci")
                    nc.sync.dma_start(out=srci[:, 0:Cw * 8],
                                      in_=srcg_in[:, woff8:woff8 + Cw * 8])
                    ohT_t = work.tile([P, CMAX * P], dt.bfloat16, tag="ohT")
                    nc.sync.dma_start(out=ohT_t[:, 0:Cw * P],
                                      in_=ohT_in[:, woff * P:(woff + Cw) * P])
                    dloc_t = work.tile([P, CMAX], dt.bfloat16, tag="dloc")
                    nc.sync.dma_start(out=dloc_t[:, 0:Cw],
                                      in_=dloc_in[:, woff:woff + Cw])

                    do_main = stage not in (23,) and not (stage == 21 and w > 0)
                    do_er = stage not in (22,) and not (stage == 21 and w > 0)
                    gath = work.tile([P, CMAX, TBL_W], dt.bfloat16, tag="gath")
                    if do_main:
                        nc.gpsimd.dma_gather(
                            out_ap=gath[:, 0:Cw, :],
                            in_ap=hall[:],
                            idxs_ap=srci[:, 0:Cw * 8],
                            num_idxs=L, num_idxs_reg=L,
                            elem_size=TBL_W, single_packet=False,
                            queue_num=(2 * w) % 4,
                        )
                    else:
                        nc.vector.memset(gath[:, 0:Cw, :], 0.0)
                    # er per edge via transposed one-hot on PE
                    erps = psum.tile([P, CMAX * 8], dt.float32, tag="erps")
                    for c in range(Cw):
                        nc.tensor.matmul(
                            out=erps[:, c * 8:(c + 1) * 8],
                            lhsT=ohT_t[:, c * P:(c + 1) * P],
                            rhs=er_own[layer][:, w * 8:(w + 1) * 8],
                            start=True, stop=True)
                    ers = work.tile([P, CMAX * 8], dt.bfloat16, tag="ers")
                    nc.scalar.copy(out=ers[:, 0:Cw * 8], in_=erps[:, 0:Cw * 8])

                    if _stage < 3:
                        touch = work.tile([P, 64], dt.float32, tag="touch")
                        nc.vector.tensor_copy(out=touch[:], in_=gath[:, 0, 0:64])
                        nc.vector.tensor_tensor(out=touch[:], in0=touch[:],
                                                in1=ers[:, 0:64], op=alu.add)
                        nc.sync.dma_start(out=out_ext[base:base + P, 0:64],
                                          in_=touch[:])
                        woff += Cw
                        woff8 += Cw * 8
                        continue

                    # ex = exp(leaky_relu(el_src + er_dst)) written into gath el slot
                    epre = work.tile([P, CMAX * 8], dt.float32, tag="epre")
                    ep = epre[:, 0:Cw * 8].rearrange("p (c h) -> p c h", h=8)
                    nc.vector.tensor_tensor(
                        out=ep, in0=gath[:, 0:Cw, 256:264],
                        in1=ers[:, 0:Cw * 8].rearrange("p (c h) -> p c h", h=8),
                        op=alu.add)
                    # leaky_relu(x) = max(x, NEG*x)
                    escl = work.tile([P, CMAX * 8], dt.float32, tag="escl")
                    es = escl[:, 0:Cw * 8].rearrange("p (c h) -> p c h", h=8)
                    nc.scalar.activation(out=es, in_=ep,
                                         func=act.Copy, scale=NEG)
                    nc.vector.tensor_tensor(out=ep, in0=ep, in1=es, op=alu.max)
                    nc.scalar.activation(out=gath[:, 0:Cw, 256:264], in_=ep,
                                         func=act.Exp)
                    # h *= ex (per-head broadcast over 32 dims)
                    feat4 = gath[:, 0:Cw, 0:HD].rearrange("p c (h d) -> p c h d", d=D)
                    exb = gath[:, 0:Cw, 256:264].unsqueeze(3).to_broadcast(
                        [P, Cw, 8, D])
                    nc.vector.tensor_tensor(out=feat4, in0=feat4, in1=exb,
                                            op=alu.mult)

                    agg = psum.tile([P, 264], dt.float32, tag="agg")
                    oh3 = work.tile([P, CMAX, P], dt.bfloat16, tag="oh")
                    nc.vector.tensor_tensor(
                        out=oh3[:, 0:Cw, :],
                        in0=dloc_t[:, 0:Cw].unsqueeze(2).to_broadcast([P, Cw, P]),
                        in1=iota_t[:].unsqueeze(1).to_broadcast([P, Cw, P]),
                        op=alu.is_equal)
                    for c in range(Cw):
                        nc.tensor.matmul(
                            out=agg[:], lhsT=oh3[:, c, :], rhs=gath[:, c, 0:264],
                            start=(c == 0), stop=(c == Cw - 1))

                    # normalize + bias + ELU
                    sinv = work.tile([P, 8], dt.float32, tag="sinv")
                    nc.vector.tensor_tensor(out=sinv[:], in0=agg[:, 256:264],
                                            in1=eps_t[:], op=alu.max)
                    nc.vector.reciprocal(out=sinv[:], in_=sinv[:])
                    xw = work.tile([P, HD], dt.float32, tag="xw")
                    xw4 = xw[:].rearrange("p (h d) -> p h d", d=D)
                    nc.vector.tensor_tensor(
                        out=xw4,
                        in0=agg[:, 0:HD].rearrange("p (h d) -> p h d", d=D),
                        in1=sinv[:].unsqueeze(2).to_broadcast([P, 8, D]),
                        op=alu.mult)
                    nc.vector.tensor_tensor(out=xw[:], in0=xw[:],
                                            in1=brep[layer][:], op=alu.add)
                    # elu(x) = relu(x) + (exp(min(x,0)) - 1)
                    emin = work.tile([P, HD], dt.float32, tag="emin")
                    nc.vector.tensor_tensor(out=emin[:], in0=xw[:],
                                            in1=zero_t[:], op=alu.min)
                    nc.scalar.activation(out=emin[:], in_=emin[:], func=act.Exp)
                    nc.scalar.activation(out=xw[:], in_=xw[:], func=act.Relu)
                    nc.vector.tensor_tensor(out=xw[:], in0=xw[:], in1=emin[:],
                                            op=alu.add)
                    nc.vector.tensor_tensor(out=xw[:], in0=xw[:], in1=mone_t[:],
                                            op=alu.add)

                    if layer == 0 and stage == 3:
                        nc.sync.dma_start(out=out_ext[base:base + P, :],
                                          in_=xw[:])
                    elif layer == 0:
                        xwb = work.tile([P, HD], dt.bfloat16, tag="xwb")
                        nc.scalar.copy(out=xwb[:], in_=xw[:])
                        for half, x2T in ((0, x2T_a), (1, x2T_b)):
                            tp = psum.tile([P, P], dt.bfloat16, tag="tp")
                            nc.tensor.transpose(
                                out=tp[:], in_=xwb[:, half * P:(half + 1) * P],
                                identity=ident_t[:])
                            nc.scalar.copy(out=x2T[:, base:base + P],
                                           in_=tp[:])
                    else:
                        nc.sync.dma_start(out=out_ext[base:base + P, :],
                                          in_=xw[:])
                    woff += Cw
                    woff8 += Cw * 8

    nc.compile()
    return nc


def _run(inputs, trace=False):
    from concourse.bass_utils import run_bass_kernel_spmd

    in_maps, C_ws = _host_prep(**inputs)
    key = C_ws
    if key not in _BUILD_CACHE:
        _BUILD_CACHE[key] = _build(C_ws)
    nc = _BUILD_CACHE[key]
    r = run_bass_kernel_spmd(nc, in_maps, core_ids=list(range(NCORES)),
                             trace=trace)
    out = np.empty((N, HD), np.float32)
    for m in range(NCORES):
        out[m * BLK:(m + 1) * BLK] = r.results[m]["out"][:BLK]
    return out, r.exec_time_ns


def kernel(**inputs):
    inputs = {k: np.asarray(v) for k, v in inputs.items()}
    out, _ = _run(inputs, trace=False)
    return out


# revision 16
# speedup vs baseline: 1.7048x; 1.0716x over previous
"""2-layer GAT (DGL GATConv eval-mode) on 8 Trainium2 NeuronCores.

Strategy:
  - Nodes are partitioned into 8 blocks of 2500 (padded to 2560 = 20 windows
    of 128). Core m owns block m: it computes the dense projections for its
    own nodes and aggregates messages for destination nodes in its block.
  - Per layer:
    Phase A (dense, sharded): h|el|er = x_blk @ [W | W@al | W@ar] via
      TensorE (bf16), written as a 384-col bf16 node table
      [h(256) | el(8) | er(8) | pad(112)] -> 768B rows; AllGather -> full
      table on every core.
    Phase B (edges, dst-sharded): edges sorted by dst, grouped into
      128-dst windows, chunks of 128 edges. Per window: one dma_gather of
      full 768B rows by src (gives h and el per edge), one 256B dma_gather
      of [el|er|pad] by local dst (gives er per edge; reads the local
      pre-AG table). ex = exp(leaky_relu(el_src + er_dst)). One-hot
      (iota == dstloc) matmuls accumulate both sum(ex*h) and sum(ex) into
      PSUM in a single 264-wide matmul per 128-edge chunk. Normalize by
      1/sum(ex), add bias, ELU.
  - Layer-1 outputs are PE-transposed into an SBUF-resident x2T so layer 2's
    dense phase needs no DMA for activations.
"""
import sys

for _p in ("/opt/trn_rl_repo",):
    if _p not in sys.path:
        sys.path.append(_p)

import numpy as np

N = 20000
E = 320000
H = 8
D = 32
HD = H * D          # 256
NEG = 0.2
P = 128
NCORES = 8
BLK = 2500          # real nodes per core block
NWIN = 20           # windows per core
BP = NWIN * P       # padded block stride 2560
NB = NCORES * BP    # padded global rows 20480
TBL_W = 384         # table row: h 0:256 | el 256:264 | er 264:272 | pad
ELR_OFF = 256       # er-gather reads cols [256:384]; er at relative 8:16

_BUILD_CACHE = {}


def _host_prep(features, W0, al0, ar0, b0, W1, al1, ar1, b1, src, dst):
    import ml_dtypes
    bf16 = ml_dtypes.bfloat16

    def wcat(W, al, ar):
        Wal = np.stack([W[:, h * D:(h + 1) * D] @ al[h] for h in range(H)], axis=1)
        War = np.stack([W[:, h * D:(h + 1) * D] @ ar[h] for h in range(H)], axis=1)
        return np.ascontiguousarray(
            np.concatenate([W, Wal, War], axis=1).astype(bf16))  # [256, 272]

    wc0 = wcat(W0, al0, ar0)
    wc1 = wcat(W1, al1, ar1)
    b0r = np.ascontiguousarray(np.tile(b0[None, :], (P, 1)).astype(np.float32))
    b1r = np.ascontiguousarray(np.tile(b1[None, :], (P, 1)).astype(np.float32))
    iota = np.ascontiguousarray(
        np.tile(np.arange(P, dtype=np.float32)[None, :], (P, 1)).astype(bf16))
    ident = np.eye(P, dtype=np.float32).astype(bf16)

    # per-core transposed, block-padded features [256, BP]
    xT = np.zeros((NCORES, HD, BP), dtype=bf16)
    xf = features.astype(bf16)
    for m in range(NCORES):
        xT[m, :, :BLK] = xf[m * BLK:(m + 1) * BLK].T

    # ---- edge partitioning ----
    owner = dst // BLK
    dst_local = dst - owner * BLK          # [0, 2500)
    win = dst_local // P                   # [0, 20)
    src_blocked = (src // BLK) * BP + (src % BLK)

    # per (core, window) edge lists
    counts = np.zeros((NCORES, NWIN), np.int64)
    lists = [[None] * NWIN for _ in range(NCORES)]
    for m in range(NCORES):
        sel = np.nonzero(owner == m)[0]
        w_m = win[sel]
        order = np.argsort(w_m, kind="stable")
        sel = sel[order]
        w_m = w_m[order]
        bounds = np.searchsorted(w_m, np.arange(NWIN + 1))
        for w in range(NWIN):
            ee = sel[bounds[w]:bounds[w + 1]]
            lists[m][w] = ee
            counts[m, w] = len(ee)

    C_ws = [int(np.ceil(counts[:, w].max() / P)) for w in range(NWIN)]
    CT = int(np.sum(C_ws))
    GC8 = CT * 8

    srcg = np.zeros((NCORES, P, GC8), np.int16)
    dloc = np.full((NCORES, P, CT), -1.0, np.float32)
    ohT = np.zeros((NCORES, P, CT * P), bf16)

    for m in range(NCORES):
        woff = 0
        woff8 = 0
        for w in range(NWIN):
            Cw = C_ws[w]
            L = Cw * P
            ee = lists[m][w]
            sb = np.zeros(L, np.int16)
            dl = np.full(L, -1.0, np.float32)
            n = len(ee)
            sb[:n] = src_blocked[ee]
            dl[:n] = (dst_local[ee] - w * P).astype(np.float32)
            # gather idx layout: index i at [i%16, i//16], replicated x8 down
            sg = sb.reshape(Cw * 8, 16).T      # [16, Cw*8]
            srcg[m, :, woff8:woff8 + Cw * 8] = np.tile(sg, (8, 1))
            # dstloc layout: edge i at [i%128, i//128]
            dloc[m, :, woff:woff + Cw] = dl.reshape(Cw, P).T
            # transposed one-hot: ohT[d, (woff+c)*128 + j] = (dstloc(edge c*128+j) == d)
            valid = np.nonzero(dl >= 0)[0]
            ohT[m, dl[valid].astype(np.int64), woff * P + valid] = 1.0
            woff += Cw
            woff8 += Cw * 8

    in_maps = []
    for m in range(NCORES):
        in_maps.append({
            "xT": np.ascontiguousarray(xT[m]),
            "wc0": wc0, "wc1": wc1, "b0r": b0r, "b1r": b1r,
            "iota": iota, "ident": ident,
            "srcg": np.ascontiguousarray(srcg[m]),
            "ohT": np.ascontiguousarray(ohT[m]),
            "dloc": np.ascontiguousarray(dloc[m].astype(bf16)),
        })
    return in_maps, tuple(C_ws)


def _build(C_ws, stage=99):
    # stage for HW bisection: 1=phaseA+AG only, 2=+gathers, 3=+edge compute,
    # 4=+transposes (full layer0), 99=full 2 layers
    import concourse.bass as bass
    import concourse.bacc as bacc
    import concourse.mybir as mybir
    import concourse.tile as tile

    dt = mybir.dt
    alu = mybir.AluOpType
    act = mybir.ActivationFunctionType

    CT = int(np.sum(C_ws))
    GC8 = CT * 8
    CMAX = max(C_ws)

    nc = bacc.Bacc("TRN2", target_bir_lowering=False, debug=False,
                   num_devices=NCORES, num_swdge_queues=4)

    xT_in = nc.dram_tensor("xT", [HD, BP], dt.bfloat16, kind="ExternalInput")
    wc0_in = nc.dram_tensor("wc0", [HD, 272], dt.bfloat16, kind="ExternalInput")
    wc1_in = nc.dram_tensor("wc1", [HD, 272], dt.bfloat16, kind="ExternalInput")
    b0r_in = nc.dram_tensor("b0r", [P, HD], dt.float32, kind="ExternalInput")
    b1r_in = nc.dram_tensor("b1r", [P, HD], dt.float32, kind="ExternalInput")
    iota_in = nc.dram_tensor("iota", [P, P], dt.bfloat16, kind="ExternalInput")
    ident_in = nc.dram_tensor("ident", [P, P], dt.bfloat16, kind="ExternalInput")
    srcg_in = nc.dram_tensor("srcg", [P, GC8], dt.int16, kind="ExternalInput")
    ohT_in = nc.dram_tensor("ohT", [P, CT * P], dt.bfloat16, kind="ExternalInput")
    dloc_in = nc.dram_tensor("dloc", [P, CT], dt.bfloat16, kind="ExternalInput")
    out_ext = nc.dram_tensor("out", [BP, HD], dt.float32, kind="ExternalOutput")

    gsems = [nc.alloc_semaphore(f"gsem{i}") for i in range(2 * NWIN)]

    with tile.TileContext(nc) as tc:
        with (
            tc.tile_pool(name="consts", bufs=1) as consts,
            tc.tile_pool(name="work", bufs=2) as work,
            tc.tile_pool(name="psum", bufs=2, space="PSUM") as psum,
            tc.tile_pool(name="dram", bufs=1, space="DRAM") as dram,
        ):
            # ---------- constants ----------
            wc_a = [consts.tile([P, 272], dt.bfloat16, name=f"wc_a{l}") for l in range(2)]
            wc_b = [consts.tile([P, 272], dt.bfloat16, name=f"wc_b{l}") for l in range(2)]
            for l, w_in in enumerate((wc0_in, wc1_in)):
                nc.sync.dma_start(out=wc_a[l][:], in_=w_in[0:P, :])
                nc.sync.dma_start(out=wc_b[l][:], in_=w_in[P:HD, :])
            brep = [consts.tile([P, HD], dt.float32, name=f"brep{l}") for l in range(2)]
            nc.sync.dma_start(out=brep[0][:], in_=b0r_in[:, :])
            nc.sync.dma_start(out=brep[1][:], in_=b1r_in[:, :])
            iota_t = consts.tile([P, P], dt.bfloat16)
            nc.sync.dma_start(out=iota_t[:], in_=iota_in[:, :])
            ident_t = consts.tile([P, P], dt.bfloat16)
            nc.sync.dma_start(out=ident_t[:], in_=ident_in[:, :])
            x2T_a = consts.tile([P, BP], dt.bfloat16)
            x2T_b = consts.tile([P, BP], dt.bfloat16)
            eps_t = consts.tile([P, 8], dt.float32)
            nc.vector.memset(eps_t[:], 1e-12)
            zero_t = consts.tile([P, HD], dt.float32)
            nc.vector.memset(zero_t[:], 0.0)
            mone_t = consts.tile([P, HD], dt.float32)
            nc.vector.memset(mone_t[:], -1.0)
            er_own = [consts.tile([P, NWIN * 8], dt.bfloat16, name=f"er_own{l}")
                      for l in range(2)]

            hel_loc = [dram.tile([BP, TBL_W], dt.bfloat16, name=f"hel_loc{l}")
                       for l in range(2)]
            hel_all = [dram.tile([NB, TBL_W], dt.bfloat16, addr_space="Shared",
                                 name=f"hel_all{l}") for l in range(2)]

            for layer in range(1 if stage < 99 else 2):
                wa, wb = wc_a[layer], wc_b[layer]
                hloc, hall = hel_loc[layer], hel_all[layer]

                # ---------- Phase A: dense projections for own block ----------
                for t in range(NWIN):
                    cs = t * P
                    if layer == 0:
                        lhs_a = work.tile([P, P], dt.bfloat16, tag="lhs_a")
                        lhs_b = work.tile([P, P], dt.bfloat16, tag="lhs_b")
                        nc.sync.dma_start(out=lhs_a[:], in_=xT_in[0:P, cs:cs + P])
                        nc.sync.dma_start(out=lhs_b[:], in_=xT_in[P:HD, cs:cs + P])
                        la, lb = lhs_a[:], lhs_b[:]
                    else:
                        la, lb = x2T_a[:, cs:cs + P], x2T_b[:, cs:cs + P]
                    h_ps = psum.tile([P, 272], dt.float32, tag="h_ps")
                    nc.tensor.matmul(out=h_ps[:], lhsT=la, rhs=wa[:],
                                     start=True, stop=False)
                    nc.tensor.matmul(out=h_ps[:], lhsT=lb, rhs=wb[:],
                                     start=False, stop=True)
                    helrow = work.tile([P, TBL_W], dt.bfloat16, tag="helrow")
                    nc.scalar.copy(out=helrow[:, 0:272], in_=h_ps[:])
                    nc.scalar.copy(out=er_own[layer][:, t * 8:(t + 1) * 8],
                                   in_=h_ps[:, 264:272])
                    nc.vector.memset(helrow[:, 272:TBL_W], 0.0)
                    nc.sync.dma_start(out=hloc[cs:cs + P, :], in_=helrow[:])

                nc.gpsimd.collective_compute(
                    "AllGather",
                    mybir.AluOpType.bypass,
                    replica_groups=[list(range(NCORES))],
                    ins=[hloc[:].opt()],
                    outs=[hall[:].opt()],
                )

                # ---------- Phase B: edge aggregation per dst window ----------
                if stage < 2:
                    continue
                _stage = 2 if stage in (21, 22, 23) else stage
                woff = 0
                woff8 = 0
                for w in range(NWIN):
                    Cw = C_ws[w]
                    L = Cw * P
                    base = w * P

                    srci = work.tile([P, CMAX * 8], dt.int16, tag="srci", bufs=3)
                    nc.sync.dma_start(out=srci[:, 0:Cw * 8],
                                      in_=srcg_in[:, woff8:woff8 + Cw * 8])
                    ohT_t = work.tile([P, CMAX * P], dt.bfloat16, tag="ohT", bufs=3)
                    nc.sync.dma_start(out=ohT_t[:, 0:Cw * P],
                                      in_=ohT_in[:, woff * P:(woff + Cw) * P])
                    dloc_t = work.tile([P, CMAX], dt.bfloat16, tag="dloc", bufs=3)
                    nc.sync.dma_start(out=dloc_t[:, 0:Cw],
                                      in_=dloc_in[:, woff:woff + Cw])

                    gath = work.tile([P, CMAX, TBL_W], dt.bfloat16, tag="gath",
                                     bufs=3)
                    nc.gpsimd.dma_gather(
                        out_ap=gath[:, 0:Cw, :],
                        in_ap=hall[:],
                        idxs_ap=srci[:, 0:Cw * 8],
                        num_idxs=L, num_idxs_reg=L,
                        elem_size=TBL_W, single_packet=False,
                        queue_num=w % 4,
                    )
                    # er per edge via transposed one-hot on PE
                    erps = psum.tile([P, CMAX * 8], dt.float32, tag="erps")
                    for c in range(Cw):
                        nc.tensor.matmul(
                            out=erps[:, c * 8:(c + 1) * 8],
                            lhsT=ohT_t[:, c * P:(c + 1) * P],
                            rhs=er_own[layer][:, w * 8:(w + 1) * 8],
                            start=True, stop=True)
                    ers = work.tile([P, CMAX * 8], dt.bfloat16, tag="ers")
                    nc.scalar.copy(out=ers[:, 0:Cw * 8], in_=erps[:, 0:Cw * 8])

                    if _stage < 3:
                        touch = work.tile([P, 64], dt.float32, tag="touch")
                        nc.vector.tensor_copy(out=touch[:], in_=gath[:, 0, 0:64])
                        nc.vector.tensor_tensor(out=touch[:], in0=touch[:],
                                                in1=ers[:, 0:64], op=alu.add)
                        nc.sync.dma_start(out=out_ext[base:base + P, 0:64],
                                          in_=touch[:])
                        woff += Cw
                        woff8 += Cw * 8
                        continue

                    # ex = exp(leaky_relu(el_src + er_dst)) written into gath el slot
                    epre = work.tile([P, CMAX * 8], dt.float32, tag="epre")
                    ep = epre[:, 0:Cw * 8].rearrange("p (c h) -> p c h", h=8)
                    nc.vector.tensor_tensor(
                        out=ep, in0=gath[:, 0:Cw, 256:264],
                        in1=ers[:, 0:Cw * 8].rearrange("p (c h) -> p c h", h=8),
                        op=alu.add)
                    # leaky_relu(x) = max(x, NEG*x)
                    escl = work.tile([P, CMAX * 8], dt.float32, tag="escl")
                    es = escl[:, 0:Cw * 8].rearrange("p (c h) -> p c h", h=8)
                    nc.scalar.activation(out=es, in_=ep,
                                         func=act.Copy, scale=NEG)
                    nc.vector.tensor_tensor(out=ep, in0=ep, in1=es, op=alu.max)
                    nc.scalar.activation(out=gath[:, 0:Cw, 256:264], in_=ep,
                                         func=act.Exp)
                    # h *= ex (per-head broadcast over 32 dims)
                    feat4 = gath[:, 0:Cw, 0:HD].rearrange("p c (h d) -> p c h d", d=D)
                    exb = gath[:, 0:Cw, 256:264].unsqueeze(3).to_broadcast(
                        [P, Cw, 8, D])
                    nc.vector.tensor_tensor(out=feat4, in0=feat4, in1=exb,
                                            op=alu.mult)

                    agg = psum.tile([P, 264], dt.float32, tag="agg")
                    oh3 = work.tile([P, CMAX, P], dt.bfloat16, tag="oh")
                    nc.vector.tensor_tensor(
                        out=oh3[:, 0:Cw, :],
                        in0=dloc_t[:, 0:Cw].unsqueeze(2).to_broadcast([P, Cw, P]),
                        in1=iota_t[:].unsqueeze(1).to_broadcast([P, Cw, P]),
                        op=alu.is_equal)
                    for c in range(Cw):
                        nc.tensor.matmul(
                            out=agg[:], lhsT=oh3[:, c, :], rhs=gath[:, c, 0:264],
                            start=(c == 0), stop=(c == Cw - 1))

                    # normalize + bias + ELU
                    sinv = work.tile([P, 8], dt.float32, tag="sinv")
                    nc.vector.tensor_tensor(out=sinv[:], in0=agg[:, 256:264],
                                            in1=eps_t[:], op=alu.max)
                    nc.vector.reciprocal(out=sinv[:], in_=sinv[:])
                    xw = work.tile([P, HD], dt.float32, tag="xw")
                    xw4 = xw[:].rearrange("p (h d) -> p h d", d=D)
                    nc.vector.tensor_tensor(
                        out=xw4,
                        in0=agg[:, 0:HD].rearrange("p (h d) -> p h d", d=D),
                        in1=sinv[:].unsqueeze(2).to_broadcast([P, 8, D]),
                        op=alu.mult)
                    nc.vector.tensor_tensor(out=xw[:], in0=xw[:],
                                            in1=brep[layer][:], op=alu.add)
                    # elu(x) = relu(x) + (exp(min(x,0)) - 1)
                    emin = work.tile([P, HD], dt.float32, tag="emin")
                    nc.vector.tensor_tensor(out=emin[:], in0=xw[:],
                                            in1=zero_t[:], op=alu.min)
                    nc.scalar.activation(out=emin[:], in_=emin[:], func=act.Exp)
                    nc.scalar.activation(out=xw[:], in_=xw[:], func=act.Relu)
                    nc.vector.tensor_tensor(out=xw[:], in0=xw[:], in1=emin[:],
                                            op=alu.add)
                    nc.vector.tensor_tensor(out=xw[:], in0=xw[:], in1=mone_t[:],
                                            op=alu.add)

                    if layer == 0 and stage == 3:
                        nc.sync.dma_start(out=out_ext[base:base + P, :],
                                          in_=xw[:])
                    elif layer == 0:
                        xwb = work.tile([P, HD], dt.bfloat16, tag="xwb")
                        nc.scalar.copy(out=xwb[:], in_=xw[:])
                        for half, x2T in ((0, x2T_a), (1, x2T_b)):
                            tp = psum.tile([P, P], dt.bfloat16, tag="tp")
                            nc.tensor.transpose(
                                out=tp[:], in_=xwb[:, half * P:(half + 1) * P],
                                identity=ident_t[:])
                            nc.scalar.copy(out=x2T[:, base:base + P],
                                           in_=tp[:])
                    else:
                        nc.sync.dma_start(out=out_ext[base:base + P, :],
                                          in_=xw[:])
                    woff += Cw
                    woff8 += Cw * 8

    nc.compile()
    return nc


def _run(inputs, trace=False):
    from concourse.bass_utils import run_bass_kernel_spmd

    in_maps, C_ws = _host_prep(**inputs)
    key = C_ws
    if key not in _BUILD_CACHE:
        _BUILD_CACHE[key] = _build(C_ws)
    nc = _BUILD_CACHE[key]
    r = run_bass_kernel_spmd(nc, in_maps, core_ids=list(range(NCORES)),
                             trace=trace)
    out = np.empty((N, HD), np.float32)
    for m in range(NCORES):
        out[m * BLK:(m + 1) * BLK] = r.results[m]["out"][:BLK]
    return out, r.exec_time_ns


def kernel(**inputs):
    inputs = {k: np.asarray(v) for k, v in inputs.items()}
    out, _ = _run(inputs, trace=False)
    return out
